# revision 28
# baseline (speedup 1.0000x reference)
"""Trainium2 Bass kernel for nn_AffinityPredictor (2-layer GCN + mean-pool + FC).

Contract: kernel(**inputs) takes the FULL unsharded inputs (as produced by
reference.setup_inputs()) and returns the FULL [1024] output.

Strategy (8 NeuronCores, SPMD — one program, per-core data):
  * Graph-parallel sharding: core c owns graphs [128c, 128(c+1)) and hence a
    contiguous node range (batch is sorted); it owns all edges whose dst falls
    in that range.  Weights + x are replicated.
  * All nodes get a single "unified" row id R (core-major, partition-major
    within a core, padded to 12800 rows/core).  Node features live in HBM
    tables addressed by R — layer 1: dinv * (x @ W1) computed densely on every
    core; layer 2: the layer-1 activations, exchanged with an 8-core
    AllGather.  Tables are stored as bf16 pair-rows [R/2, 128] (two nodes per
    256-byte row) so the MoE bulk-gather primitive (dma_gather, int16 indices,
    256B elements) can fetch per-edge rows; indices are split in two 32k-row
    slabs to satisfy int16.
  * The scatter/segment-sum side is eliminated: the host sorts each core's
    edges by destination into windows of 32 consecutive dst nodes (padded to
    a uniform per-slab tile count so one program serves all cores) and ships
    binary bf16 one-hot tiles (lo/hi pair-half split); the segment sum becomes
    PE matmuls accumulating into a per-window [32, 64] PSUM tile.
  * GCN normalization D^-1/2 (A+I) D^-1/2 is folded in as activation scales
    (dinv at the table build, dinv(dst) — squared for layer 1 — at the window
    flush) and a rank-1 bias matmul with lhsT = sqrt(deg); relu commutes with
    the positive dinv scale.
  * Mean pooling via binary one-hot matmuls; 1/count and the fc bias are
    applied to the final [1, 128] result.

All index/structure preprocessing (degrees, sorting, padding, one-hots)
happens on the host in numpy; every FLOP on x/W data runs on device.
"""

import numpy as np
import ml_dtypes

import concourse.tile as tile
from concourse import bass, bacc, mybir
from concourse.bass_utils import run_bass_kernel_spmd

# ---------------------------------------------------------------- constants
N_NODES = 100_000
NUM_GRAPHS = 1024
IN_DIM = 20
NODE_DIM = 64
N_CORES = 8
GRAPHS_PER_CORE = NUM_GRAPHS // N_CORES      # 128
P = 128

NPAD = 12_800                                # padded nodes per core
NT = NPAD // P                               # node tiles per core = 100
WIN = 32                                     # dst nodes per window
NW = NPAD // WIN                             # windows per core = 400
WPG = P // WIN                               # windows per node tile = 4
NW3 = (NW + 2) // 3

NROWS = N_CORES * NPAD                       # unified rows = 102400
NPAIR = NROWS // 2                           # pair rows = 51200
SLAB = 32_768                                # pair rows per index slab
NSLAB = 2                                    # 51200 -> slabs [32768, 18432]

# x packing for the dense x@W1 stage: 3 groups of 20 features at partition
# bases {0, 32, 64}; group width covers unified rows (padded to 128 tiles).
XGROUPS = 3
XG_W = 34_176                                # 267 tiles of 128
XG_TILES = XG_W // P                         # 267
XT_TILES = XGROUPS * XG_TILES                # 801
TAB1_ROWS = P * XT_TILES                     # 102528 >= NROWS
XCHUNK = 45                                  # x tiles per streamed chunk (801=45*17+36)

BF16 = mybir.dt.bfloat16
F32 = mybir.dt.float32
I16 = mybir.dt.int16

_CACHE = {}


# ================================================================ host prep
def _preprocess(x, edge_index, batch, W1, b1, W2, b2, Wfc, bfc):
    x = np.asarray(x, np.float32)
    edge_index = np.asarray(edge_index, np.int64)
    batch = np.asarray(batch, np.int64)

    n = N_NODES
    loop = np.arange(n, dtype=np.int64)
    src = np.concatenate([edge_index[0], loop])
    dst = np.concatenate([edge_index[1], loop])

    deg = np.bincount(dst, minlength=n).astype(np.float32)
    dinv = np.where(deg > 0, 1.0 / np.sqrt(deg), 0.0).astype(np.float32)
    sdeg = np.where(deg > 0, np.sqrt(deg), 0.0).astype(np.float32)

    gbound = np.searchsorted(batch, np.arange(0, NUM_GRAPHS + 1, GRAPHS_PER_CORE))
    n0s, n1s = gbound[:-1], gbound[1:]

    core_of = np.searchsorted(gbound[1:], np.arange(n), side="right")
    local_of = np.arange(n) - n0s[core_of]
    # unified row id: core-major, partition-major within core
    R_of = core_of * NPAD + (local_of % P) * NT + local_of // P

    # inverse map: unified row r -> node id (or -1 for pad rows)
    node_of_R = np.full(NROWS, -1, np.int64)
    node_of_R[R_of] = np.arange(n)

    # ---- per-core edge partitioning & window packing (slab-split tiles)
    src_pair = R_of[src] // 2
    src_half = (R_of[src] % 2).astype(np.int64)
    src_slab = (src_pair // SLAB).astype(np.int64)

    edst_core = core_of[dst]
    per_core = []
    max_cnt = np.zeros(NSLAB, np.int64)
    for c in range(N_CORES):
        m = edst_core == c
        s_pair, s_half, s_slab, d_c = src_pair[m], src_half[m], src_slab[m], dst[m]
        ld = (d_c - n0s[c]).astype(np.int64)
        # sort so (window, slab) groups are contiguous (pos packing relies on it)
        order = np.lexsort((s_pair, s_slab, ld // WIN))
        s_pair, s_half, s_slab, ld = (s_pair[order], s_half[order],
                                      s_slab[order], ld[order])
        w = ld // WIN
        for s in range(NSLAB):
            cnts = np.bincount(w[s_slab == s], minlength=NW)
            max_cnt[s] = max(max_cnt[s], int(cnts.max()))
        per_core.append((s_pair, s_half, s_slab, ld))

    TPWs = [int(np.ceil(mc / P)) for mc in max_cnt]   # tiles/window per slab
    TPW_ALL = int(sum(TPWs))                          # tiles per window total
    T_TOTAL = NW * TPW_ALL
    SLOTS = T_TOTAL * P

    TPG = WPG * TPW_ALL
    in_maps = []
    for c in range(N_CORES):
        s_pair, s_half, s_slab, ld = per_core[c]
        w = ld // WIN
        # slot layout: group (= node tile = 4 windows) major; within a group:
        # [slab0: window-major tiles][slab1: window-major tiles]; dense pack
        # per (window, slab).
        key = w * NSLAB + s_slab
        cnts_ws = np.bincount(key, minlength=NW * NSLAB)
        starts = np.zeros(NW * NSLAB, np.int64)
        starts[1:] = np.cumsum(cnts_ws)[:-1]
        pos = np.arange(len(ld)) - starts[key]
        g_ = w // WPG
        b_ = w % WPG
        base_T = g_ * TPG + np.where(s_slab == 0, 0, WPG * TPWs[0]) + \
            b_ * np.where(s_slab == 0, TPWs[0], TPWs[1] if NSLAB > 1 else 0)
        slot = base_T * P + pos

        pidx = np.zeros(SLOTS, np.int64)              # pair row within slab
        oneh = np.zeros((SLOTS, 2 * WIN), ml_dtypes.bfloat16)
        pidx[slot] = s_pair - s_slab * SLAB
        oneh[slot, (ld % WIN) + WIN * s_half] = 1.0

        # idx device layout: per (group, slab) contiguous block,
        # 16-partition wrap, replicated to 128 partitions
        idx_dev = np.zeros((P, T_TOTAL * P // 16), np.int16)
        col = 0
        pidx_t = pidx.reshape(T_TOTAL, P)
        for g in range(NT):
            for s in range(NSLAB):
                t0 = g * TPG + (0 if s == 0 else WPG * TPWs[0])
                blk = pidx_t[t0:t0 + WPG * TPWs[s]].reshape(-1)
                nb = blk.size // 16
                idx_dev[:16, col:col + nb] = blk.reshape(nb, 16).T
                col += nb
        idx_dev[16:, :] = np.tile(idx_dev[:16, :], (7, 1))

        oneh = np.ascontiguousarray(
            oneh.reshape(T_TOTAL, P, 2 * WIN).transpose(1, 0, 2)
        ).reshape(P, T_TOTAL * 2 * WIN)

        # ---- per-node scale vectors, local layout (node l = 128 t + p)
        n_real = int(n1s[c] - n0s[c])
        l_arr = np.arange(NPAD)
        gl = np.minimum(n0s[c] + l_arr, n - 1)
        valid = l_arr < n_real
        dinv_l = np.where(valid, dinv[gl], 0.0).astype(np.float32)
        sdeg_l = np.where(valid, sdeg[gl], 0.0).astype(np.float32)
        dinvp = np.ascontiguousarray(dinv_l.reshape(NT, P).T)
        dinv2p = np.ascontiguousarray((dinv_l ** 2).reshape(NT, P).T)
        sdeg3 = np.zeros((65, NW3 * WIN), np.float32)
        sw = sdeg_l.reshape(NW, WIN)
        for r in range(3):
            rows = sw[r::3]
            sdeg3[32 * r, :rows.shape[0] * WIN] = rows.reshape(-1)

        # ---- pooling one-hot (binary)
        cnt_g = np.bincount((batch[n0s[c]:n1s[c]] - c * GRAPHS_PER_CORE).astype(np.int64),
                            minlength=GRAPHS_PER_CORE).astype(np.float32)
        invc = (1.0 / np.maximum(cnt_g, 1.0)).astype(np.float32).reshape(1, GRAPHS_PER_CORE)
        lg = (batch[gl] - c * GRAPHS_PER_CORE).astype(np.int64)
        poolh = np.zeros((NPAD, GRAPHS_PER_CORE), ml_dtypes.bfloat16)
        poolh[l_arr[valid], lg[valid]] = 1.0
        poolh = np.ascontiguousarray(
            poolh.reshape(NT, P, GRAPHS_PER_CORE).transpose(1, 0, 2)
        ).reshape(P, NT * GRAPHS_PER_CORE)

        in_maps.append({
            "gidx": idx_dev, "oneh": oneh, "poolh": poolh,
            "dinvp": dinvp, "dinv2p": dinv2p, "sdeg3": sdeg3, "invc": invc,
        })

    # ---- replicated tensors: xpack in unified-row order
    # xstage tile (g, j) partition p -> table1 row r = p*XT_TILES + g*XG_TILES + j
    # xpack column (g, j*128 + p) holds node_of_R(r) features (or 0)
    rows_grid = np.arange(P * XT_TILES).reshape(P, XT_TILES)
    p1 = rows_grid // XT_TILES
    i1 = rows_grid % XT_TILES
    # r for (p, i):
    r_of_pi = p1 * XT_TILES + i1   # == rows_grid... identity by construction
    # node for table1 row r (r may exceed NROWS for pad tiles)
    node_pi = np.where(r_of_pi < NROWS, node_of_R[np.minimum(r_of_pi, NROWS - 1)], -1)
    dinv1p = np.where(node_pi >= 0, dinv[np.maximum(node_pi, 0)], 0.0).astype(np.float32)

    xt = np.zeros((84, XG_W), ml_dtypes.bfloat16)
    xT = x.T.astype(ml_dtypes.bfloat16)      # [20, n]
    for g in range(XGROUPS):
        for j in range(XG_TILES):
            i = g * XG_TILES + j
            nodes = node_pi[:, i]             # [128] node per partition
            ok = nodes >= 0
            colsl = slice(j * P, j * P + P)
            blk = np.zeros((IN_DIM, P), ml_dtypes.bfloat16)
            blk[:, ok] = xT[:, nodes[ok]]
            xt[g * 32:g * 32 + IN_DIM, colsl] = blk

    w1rep = np.zeros((84, NODE_DIM), ml_dtypes.bfloat16)
    b1rep = np.zeros((65, NODE_DIM), np.float32)
    b2rep = np.zeros((65, NODE_DIM), np.float32)
    w1_bf = np.asarray(W1, np.float32).astype(ml_dtypes.bfloat16)
    for r in range(3):
        w1rep[32 * r:32 * r + IN_DIM, :] = w1_bf
        b1rep[32 * r, :] = np.asarray(b1, np.float32)
        b2rep[32 * r, :] = np.asarray(b2, np.float32)

    shared = {
        "xpack": xt,
        "ident": np.eye(P, dtype=ml_dtypes.bfloat16),
        "w1rep": w1rep,
        "w2": np.asarray(W2, np.float32).astype(ml_dtypes.bfloat16),
        "wfc": np.asarray(Wfc, np.float32).astype(ml_dtypes.bfloat16),
        "b1rep": b1rep,
        "b2rep": b2rep,
        "bfc": np.full((1, GRAPHS_PER_CORE), np.float32(np.asarray(bfc).reshape(-1)[0])),
        "dinv1p": np.ascontiguousarray(dinv1p),
    }
    for m in in_maps:
        m.update(shared)
    return in_maps, tuple(TPWs)


# ============================================================= device program
def _build_program(TPWs, debug=False):
    TPW_ALL = int(sum(TPWs))
    T_TOTAL = NW * TPW_ALL
    TPG = WPG * TPW_ALL                  # tiles per node-group
    IDXC = T_TOTAL * P // 16             # idx cols total
    OH_GROUPS = 2

    nc = bacc.Bacc()
    xpack = nc.declare_dram_parameter("xpack", [84, XG_W], BF16, isOutput=False)
    ident = nc.declare_dram_parameter("ident", [P, P], BF16, isOutput=False)
    w1rep = nc.declare_dram_parameter("w1rep", [84, NODE_DIM], BF16, isOutput=False)
    w2 = nc.declare_dram_parameter("w2", [NODE_DIM, NODE_DIM], BF16, isOutput=False)
    wfc = nc.declare_dram_parameter("wfc", [NODE_DIM, 1], BF16, isOutput=False)
    b1rep = nc.declare_dram_parameter("b1rep", [65, NODE_DIM], F32, isOutput=False)
    b2rep = nc.declare_dram_parameter("b2rep", [65, NODE_DIM], F32, isOutput=False)
    bfc = nc.declare_dram_parameter("bfc", [1, GRAPHS_PER_CORE], F32, isOutput=False)
    invc = nc.declare_dram_parameter("invc", [1, GRAPHS_PER_CORE], F32, isOutput=False)
    dinv1p = nc.declare_dram_parameter("dinv1p", [P, XT_TILES], F32, isOutput=False)
    dinvp = nc.declare_dram_parameter("dinvp", [P, NT], F32, isOutput=False)
    dinv2p = nc.declare_dram_parameter("dinv2p", [P, NT], F32, isOutput=False)
    sdeg3 = nc.declare_dram_parameter("sdeg3", [65, NW3 * WIN], F32, isOutput=False)
    gidx = nc.declare_dram_parameter("gidx", [P, IDXC], I16, isOutput=False)
    oneh = nc.declare_dram_parameter("oneh", [P, T_TOTAL * 2 * WIN], BF16, isOutput=False)
    poolh = nc.declare_dram_parameter("poolh", [P, NT * GRAPHS_PER_CORE], BF16, isOutput=False)
    out = nc.declare_dram_parameter("out", [1, GRAPHS_PER_CORE], F32, isOutput=True)

    table1 = nc.dram_tensor("table1", [TAB1_ROWS, NODE_DIM], BF16)
    h1_slice = nc.dram_tensor("h1_slice", [NPAD, NODE_DIM], BF16)
    table2 = nc.dram_tensor("table2", [NROWS, NODE_DIM], BF16, addr_space="Shared")
    t1_pair = table1[:].rearrange("(q two) d -> q (two d)", two=2)   # [51264, 128]
    t2_pair = table2[:].rearrange("(q two) d -> q (two d)", two=2)   # [51200, 128]

    if debug:
        dbg_h1 = nc.declare_dram_parameter("dbg_h1", [P, NT * NODE_DIM], BF16, isOutput=True)
        dbg_h2 = nc.declare_dram_parameter("dbg_h2", [P, NT * NODE_DIM], BF16, isOutput=True)
        dbg_gt = nc.declare_dram_parameter("dbg_gt", [P, TPG * 2 * NODE_DIM], BF16, isOutput=True)
        dbg_ps = nc.declare_dram_parameter("dbg_ps", [32, NODE_DIM], F32, isOutput=True)
        dbg_pool = nc.declare_dram_parameter("dbg_pool", [NODE_DIM, GRAPHS_PER_CORE], F32, isOutput=True)

    with tile.TileContext(nc) as tc:
        with (
            tc.tile_pool(name="const", bufs=1) as constp,
            tc.tile_pool(name="idxp", bufs=3) as idxp,
            tc.tile_pool(name="xstr", bufs=2) as xstrp,
            tc.tile_pool(name="stage", bufs=2) as stagep,
            tc.tile_pool(name="gat", bufs=3) as gatp,
            tc.tile_pool(name="ohp", bufs=3) as ohp,
            tc.tile_pool(name="hsb", bufs=1) as hsbp,
            tc.tile_pool(name="php", bufs=2) as php,
            tc.tile_pool(name="psA", bufs=1, space="PSUM") as psA,
            tc.tile_pool(name="psB", bufs=3, space="PSUM") as psB,
            tc.tile_pool(name="psT", bufs=2, space="PSUM") as psT,
            tc.tile_pool(name="psC", bufs=1, space="PSUM") as psC,
        ):
            # ---------------- constants
            w1_sb = constp.tile([84, NODE_DIM], BF16)
            w2_sb = constp.tile([NODE_DIM, NODE_DIM], BF16)
            wfc_sb = constp.tile([NODE_DIM, 1], BF16)
            b1_sb = constp.tile([65, NODE_DIM], F32)
            b2_sb = constp.tile([65, NODE_DIM], F32)
            bfc_sb = constp.tile([1, GRAPHS_PER_CORE], F32)
            invc_sb = constp.tile([1, GRAPHS_PER_CORE], F32)
            dinv1_sb = constp.tile([P, XT_TILES], F32)
            dinv_sb = constp.tile([P, NT], F32)
            dinv2_sb = constp.tile([P, NT], F32)
            sdeg_sb = constp.tile([65, NW3 * WIN], F32)
            id_sb = constp.tile([P, P], BF16)
            nc.sync.dma_start(out=id_sb[:], in_=ident[:])
            for dst_t, src_t in ((w1_sb, w1rep), (w2_sb, w2), (wfc_sb, wfc),
                                 (b1_sb, b1rep), (b2_sb, b2rep), (bfc_sb, bfc),
                                 (invc_sb, invc), (dinv1_sb, dinv1p),
                                 (dinv_sb, dinvp), (dinv2_sb, dinv2p),
                                 (sdeg_sb, sdeg3)):
                nc.sync.dma_start(out=dst_t[:], in_=src_t[:])

            # -------- stage X: table1 = dinv * (x @ W1), bf16, unified rows
            t1v = table1[:].rearrange("(p t) d -> p (t d)", p=P)
            for j2 in range(0, XG_TILES, XCHUNK):
                jn = min(XCHUNK, XG_TILES - j2)
                xp_sb = xstrp.tile([84, XCHUNK * P], BF16, tag="xp")
                nc.sync.dma_start(out=xp_sb[:, :jn * P],
                                  in_=xpack[:, j2 * P:(j2 + jn) * P])
                for g in range(XGROUPS):
                    stg = stagep.tile([P, XCHUNK * NODE_DIM], BF16, tag="xstg")
                    for j3 in range(0, jn, 8):
                        j4 = min(8, jn - j3)
                        ps = psA.tile([P, 8 * NODE_DIM], F32, tag="xps")
                        for j in range(j3, j3 + j4):
                            nc.tensor.matmul(
                                out=ps[:, (j - j3) * NODE_DIM:(j - j3 + 1) * NODE_DIM],
                                lhsT=xp_sb[g * 32:g * 32 + IN_DIM, j * P:(j + 1) * P],
                                rhs=w1_sb[g * 32:g * 32 + IN_DIM, :],
                                start=True, stop=True,
                            )
                        ti = g * XG_TILES + j2 + j3
                        nc.vector.tensor_tensor(
                            out=stg[:, j3 * NODE_DIM:(j3 + j4) * NODE_DIM]
                                .rearrange("p (a b) -> p a b", b=NODE_DIM),
                            in0=ps[:, :j4 * NODE_DIM]
                                .rearrange("p (a b) -> p a b", b=NODE_DIM),
                            in1=dinv1_sb[:, ti:ti + j4].to_broadcast([P, j4, NODE_DIM]),
                            op=mybir.AluOpType.mult,
                        )
                    nc.sync.dma_start(
                        out=t1v[:, (g * XG_TILES + j2) * NODE_DIM:
                                (g * XG_TILES + j2 + jn) * NODE_DIM],
                        in_=stg[:, :jn * NODE_DIM],
                    )

            # table1 writes must complete before layer-1 gathers read it
            tc.strict_bb_all_engine_barrier()

            # ---------------- message-passing layers
            # per-group gather sizes (group = 4 windows)
            n_idx_s = [WPG * TPWs[s] * P for s in range(NSLAB)]   # idxs per (grp, slab)
            idxcols_grp = sum(n_idx_s) // 16
            h_sb = {}
            for layer in (1, 2):
                table_p = t1_pair if layer == 1 else t2_pair
                bias_sb = b1_sb if layer == 1 else b2_sb
                scale_sb = dinv2_sb if layer == 1 else dinv_sb
                h = hsbp.tile([P, NT * NODE_DIM], BF16, tag=f"h{layer}")
                h_sb[layer] = h

                for og in range(0, NT, OH_GROUPS):
                    ogn = min(OH_GROUPS, NT - og)
                    oh_sb = ohp.tile([P, OH_GROUPS * TPG * 2 * WIN], BF16, tag="oh")
                    nc.sync.dma_start(
                        out=oh_sb[:, :ogn * TPG * 2 * WIN],
                        in_=oneh[:, og * TPG * 2 * WIN:(og + ogn) * TPG * 2 * WIN],
                    )
                    ix_sb = idxp.tile([P, OH_GROUPS * idxcols_grp], I16, tag="ix")
                    nc.sync.dma_start(
                        out=ix_sb[:, :ogn * idxcols_grp],
                        in_=gidx[:, og * idxcols_grp:(og + ogn) * idxcols_grp],
                    )
                    for bg in range(og, og + ogn):
                        # gather: one call per slab; tile layout within group:
                        # [slab0: WPG*TPW0 tiles][slab1: WPG*TPW1 tiles]
                        gt = gatp.tile([P, TPG * 2 * NODE_DIM], BF16, tag="gt")
                        gtv = gt[:].rearrange("p (t r) -> p t r", r=2 * NODE_DIM)
                        colb = (bg - og) * idxcols_grp
                        tile0 = 0
                        for s in range(NSLAB):
                            nts = WPG * TPWs[s]
                            nc.gpsimd.dma_gather(
                                out_ap=gtv[:, tile0:tile0 + nts, :],
                                in_ap=table_p[s * SLAB:
                                              min((s + 1) * SLAB, table_p.shape[0]), :],
                                idxs_ap=ix_sb[:, colb:colb + n_idx_s[s] // 16],
                                num_idxs=n_idx_s[s],
                                num_idxs_reg=n_idx_s[s],
                                elem_size=2 * NODE_DIM,
                                single_packet=False,
                            )
                            colb += n_idx_s[s] // 16
                            tile0 += nts
                        for b_ in range(WPG):
                            w_glob = bg * WPG + b_
                            p3, c3 = 32 * (w_glob % 3), w_glob // 3
                            # this window's tiles: slab0 then slab1 segments
                            tlist = [b_ * TPWs[0] + q for q in range(TPWs[0])]
                            if NSLAB > 1:
                                tlist += [WPG * TPWs[0] + b_ * TPWs[1] + q
                                          for q in range(TPWs[1])]
                            ps = psB.tile([32, NODE_DIM], F32, tag="mps")
                            nc.tensor.matmul(
                                out=ps[:],
                                lhsT=sdeg_sb[p3:p3 + 1, c3 * WIN:(c3 + 1) * WIN],
                                rhs=bias_sb[p3:p3 + 1, :], start=True, stop=False)
                            for ti, t in enumerate(tlist):
                                ohb = ((bg - og) * TPG + t) * 2 * WIN
                                for hf in range(2):
                                    nc.tensor.matmul(
                                        out=ps[:],
                                        lhsT=oh_sb[:, ohb + hf * WIN:ohb + (hf + 1) * WIN],
                                        rhs=gt[:, t * 2 * NODE_DIM + hf * NODE_DIM:
                                               t * 2 * NODE_DIM + (hf + 1) * NODE_DIM],
                                        start=False,
                                        stop=(ti == len(tlist) - 1 and hf == 1),
                                    )
                            nc.scalar.activation(
                                out=h[b_ * WIN:(b_ + 1) * WIN,
                                      bg * NODE_DIM:(bg + 1) * NODE_DIM],
                                in_=ps[:], func=mybir.ActivationFunctionType.Relu,
                                scale=scale_sb[b_ * WIN:(b_ + 1) * WIN, bg:bg + 1],
                            )
                        if debug and layer == 1 and bg == 0:
                            nc.sync.dma_start(out=dbg_gt[:], in_=gt[:])

                if layer == 1:
                    # table2 rows must be dinv*(h1 @ W2): conv2 = (A_hat h1) W2
                    # commutes, so transform the slice before the AllGather.
                    # Per node-group: transpose -> W2 matmul -> transpose back.
                    h1w = hsbp.tile([P, NT * NODE_DIM], BF16, tag="h1w")
                    for g in range(NT):
                        psT1 = psT.tile([NODE_DIM, P], BF16, tag="tr")
                        nc.tensor.transpose(
                            out=psT1[:], in_=h[:, g * NODE_DIM:(g + 1) * NODE_DIM],
                            identity=id_sb[:])
                        hT = stagep.tile([NODE_DIM, P], BF16, tag="hT")
                        nc.vector.tensor_copy(out=hT[:], in_=psT1[:])
                        psT2 = psT.tile([NODE_DIM, P], F32, tag="tr")
                        nc.tensor.matmul(out=psT2[:], lhsT=w2_sb[:], rhs=hT[:],
                                         start=True, stop=True)
                        hwT = stagep.tile([NODE_DIM, P], BF16, tag="hwT")
                        nc.vector.tensor_copy(out=hwT[:], in_=psT2[:])
                        psT3 = psT.tile([P, NODE_DIM], BF16, tag="tr")
                        nc.tensor.transpose(
                            out=psT3[:], in_=hwT[:], identity=id_sb[:NODE_DIM, :NODE_DIM])
                        nc.scalar.copy(
                            out=h1w[:, g * NODE_DIM:(g + 1) * NODE_DIM], in_=psT3[:])
                    nc.sync.dma_start(
                        out=h1_slice[:].rearrange("(p t) d -> p (t d)", p=P),
                        in_=h1w[:])
                    nc.gpsimd.collective_compute(
                        "AllGather",
                        mybir.AluOpType.bypass,
                        replica_groups=[list(range(N_CORES))],
                        ins=[h1_slice[:]],
                        outs=[table2[:]],
                    )
                    tc.strict_bb_all_engine_barrier()
                    if debug:
                        nc.sync.dma_start(out=dbg_h1[:], in_=h[:])
                if layer == 2 and debug:
                    nc.sync.dma_start(out=dbg_h2[:], in_=h[:])

            # ---------------- mean pool + fc
            pool_ps = psC.tile([NODE_DIM, GRAPHS_PER_CORE], F32, tag="pps")
            PHC = 25
            for t0 in range(0, NT, PHC):
                ph_sb = php.tile([P, PHC * GRAPHS_PER_CORE], BF16, tag="ph")
                nc.sync.dma_start(
                    out=ph_sb[:],
                    in_=poolh[:, t0 * GRAPHS_PER_CORE:(t0 + PHC) * GRAPHS_PER_CORE])
                for t in range(t0, t0 + PHC):
                    nc.tensor.matmul(
                        out=pool_ps[:],
                        lhsT=h_sb[2][:, t * NODE_DIM:(t + 1) * NODE_DIM],
                        rhs=ph_sb[:, (t - t0) * GRAPHS_PER_CORE:
                                  (t - t0 + 1) * GRAPHS_PER_CORE],
                        start=(t == 0), stop=(t == NT - 1),
                    )
            pool_sb = stagep.tile([NODE_DIM, GRAPHS_PER_CORE], BF16, tag="pool")
            nc.vector.tensor_copy(out=pool_sb[:], in_=pool_ps[:])
            if debug:
                pool_f32 = stagep.tile([NODE_DIM, GRAPHS_PER_CORE], F32, tag="poolf")
                nc.vector.tensor_copy(out=pool_f32[:], in_=pool_ps[:])
                nc.sync.dma_start(out=dbg_pool[:], in_=pool_f32[:])

            fc_ps = psC.tile([1, GRAPHS_PER_CORE], F32, tag="fc")
            nc.tensor.matmul(out=fc_ps[:], lhsT=wfc_sb[:], rhs=pool_sb[:],
                             start=True, stop=True)
            out_sb = stagep.tile([1, GRAPHS_PER_CORE], F32, tag="osb")
            nc.vector.tensor_tensor(out=out_sb[:], in0=fc_ps[:], in1=invc_sb[:],
                                    op=mybir.AluOpType.mult)
            nc.vector.tensor_tensor(out=out_sb[:], in0=out_sb[:], in1=bfc_sb[:],
                                    op=mybir.AluOpType.add)
            nc.sync.dma_start(out=out[:], in_=out_sb[:])

    nc.compile()
    return nc


# ================================================================== kernel
def kernel(**inputs) -> np.ndarray:
    in_maps, TPWs = _preprocess(
        inputs["x"], inputs["edge_index"], inputs["batch"],
        inputs["W1"], inputs["b1"], inputs["W2"], inputs["b2"],
        inputs["Wfc"], inputs["bfc"],
    )
    if TPWs not in _CACHE:
        _CACHE[TPWs] = _build_program(TPWs)
    nc = _CACHE[TPWs]
    res = run_bass_kernel_spmd(nc, in_maps, list(range(N_CORES)))
    outs = [res.results[c]["out"].reshape(-1) for c in range(N_CORES)]
    return np.concatenate(outs).astype(np.float32)


# revision 34
# speedup vs baseline: 55.4805x; 55.4805x over previous
"""Trainium2 Bass kernel for nn_AffinityPredictor (2-layer GCN + mean-pool + FC).

Contract: kernel(**inputs) takes the FULL unsharded inputs (as produced by
reference.setup_inputs()) and returns the FULL [1024] output.

Strategy (8 NeuronCores, SPMD — one program, per-core data):
  * Graph-parallel sharding: core c owns graphs [128c, 128(c+1)) and hence a
    contiguous node range (batch is sorted); it owns all edges whose dst falls
    in that range.  Weights + x are replicated.
  * All nodes get a single "unified" row id R (core-major, partition-major
    within a core, padded to 12800 rows/core).  Node features live in HBM
    tables addressed by R — layer 1: dinv * (x @ W1) computed densely on every
    core; layer 2: the layer-1 activations, exchanged with an 8-core
    AllGather.  Tables are stored as bf16 pair-rows [R/2, 128] (two nodes per
    256-byte row) so the MoE bulk-gather primitive (dma_gather, int16 indices,
    256B elements) can fetch per-edge rows; indices are split in two 32k-row
    slabs to satisfy int16.
  * The scatter/segment-sum side is eliminated: the host sorts each core's
    edges by destination into windows of 32 consecutive dst nodes (padded to
    a uniform per-slab tile count so one program serves all cores) and ships
    binary bf16 one-hot tiles (lo/hi pair-half split); the segment sum becomes
    PE matmuls accumulating into a per-window [32, 64] PSUM tile.
  * GCN normalization D^-1/2 (A+I) D^-1/2 is folded in as activation scales
    (dinv at the table build, dinv(dst) — squared for layer 1 — at the window
    flush) and a rank-1 bias matmul with lhsT = sqrt(deg); relu commutes with
    the positive dinv scale.
  * Mean pooling via binary one-hot matmuls; 1/count and the fc bias are
    applied to the final [1, 128] result.

All index/structure preprocessing (degrees, sorting, padding, one-hots)
happens on the host in numpy; every FLOP on x/W data runs on device.
"""

import numpy as np
import ml_dtypes

import concourse.tile as tile
from concourse import bass, bacc, mybir
from concourse.bass_utils import run_bass_kernel_spmd

# ---------------------------------------------------------------- constants
N_NODES = 100_000
NUM_GRAPHS = 1024
IN_DIM = 20
NODE_DIM = 64
N_CORES = 8
GRAPHS_PER_CORE = NUM_GRAPHS // N_CORES      # 128
P = 128

NPAD = 12_800                                # padded nodes per core
NT = NPAD // P                               # node tiles per core = 100
WIN = 32                                     # dst nodes per window
NW = NPAD // WIN                             # windows per core = 400
WPG = P // WIN                               # windows per node tile = 4
NW3 = (NW + 2) // 3

NROWS = N_CORES * NPAD                       # unified rows = 102400
NPAIR = NROWS // 2                           # pair rows = 51200
SLAB = 32_768                                # pair rows per index slab
NSLAB = 2                                    # 51200 -> slabs [32768, 18432]

# x packing for the dense x@W1 stage: 3 groups of 20 features at partition
# bases {0, 32, 64}; group width covers unified rows (padded to 128 tiles).
XGROUPS = 3
XG_W = 34_176                                # 267 tiles of 128
XG_TILES = XG_W // P                         # 267
XT_TILES = XGROUPS * XG_TILES                # 801
TAB1_ROWS = P * XT_TILES                     # 102528 >= NROWS
XCHUNK = 45                                  # x tiles per streamed chunk (801=45*17+36)

BF16 = mybir.dt.bfloat16
F32 = mybir.dt.float32
I16 = mybir.dt.int16

_CACHE = {}


# ================================================================ host prep
def _preprocess(x, edge_index, batch, W1, b1, W2, b2, Wfc, bfc):
    x = np.asarray(x, np.float32)
    edge_index = np.asarray(edge_index, np.int64)
    batch = np.asarray(batch, np.int64)

    n = N_NODES
    loop = np.arange(n, dtype=np.int64)
    src = np.concatenate([edge_index[0], loop])
    dst = np.concatenate([edge_index[1], loop])

    deg = np.bincount(dst, minlength=n).astype(np.float32)
    dinv = np.where(deg > 0, 1.0 / np.sqrt(deg), 0.0).astype(np.float32)
    sdeg = np.where(deg > 0, np.sqrt(deg), 0.0).astype(np.float32)

    gbound = np.searchsorted(batch, np.arange(0, NUM_GRAPHS + 1, GRAPHS_PER_CORE))
    n0s, n1s = gbound[:-1], gbound[1:]

    core_of = np.searchsorted(gbound[1:], np.arange(n), side="right")
    local_of = np.arange(n) - n0s[core_of]
    # unified row id: core-major, partition-major within core
    R_of = core_of * NPAD + (local_of % P) * NT + local_of // P

    # inverse map: unified row r -> node id (or -1 for pad rows)
    node_of_R = np.full(NROWS, -1, np.int64)
    node_of_R[R_of] = np.arange(n)

    # ---- per-core edge partitioning & window packing (slab-split tiles)
    src_pair = R_of[src] // 2
    src_half = (R_of[src] % 2).astype(np.int64)
    src_slab = (src_pair // SLAB).astype(np.int64)

    edst_core = core_of[dst]
    per_core = []
    max_cnt = np.zeros(NSLAB, np.int64)
    for c in range(N_CORES):
        m = edst_core == c
        s_pair, s_half, s_slab, d_c = src_pair[m], src_half[m], src_slab[m], dst[m]
        ld = (d_c - n0s[c]).astype(np.int64)
        # sort so (window, slab) groups are contiguous (pos packing relies on it)
        order = np.lexsort((s_pair, s_slab, ld // WIN))
        s_pair, s_half, s_slab, ld = (s_pair[order], s_half[order],
                                      s_slab[order], ld[order])
        w = ld // WIN
        for s in range(NSLAB):
            cnts = np.bincount(w[s_slab == s], minlength=NW)
            max_cnt[s] = max(max_cnt[s], int(cnts.max()))
        per_core.append((s_pair, s_half, s_slab, ld))

    TPWs = [int(np.ceil(mc / P)) for mc in max_cnt]   # tiles/window per slab
    TPW_ALL = int(sum(TPWs))                          # tiles per window total
    T_TOTAL = NW * TPW_ALL
    SLOTS = T_TOTAL * P

    TPG = WPG * TPW_ALL
    in_maps = []
    for c in range(N_CORES):
        s_pair, s_half, s_slab, ld = per_core[c]
        w = ld // WIN
        # slot layout: group (= node tile = 4 windows) major; within a group:
        # [slab0: window-major tiles][slab1: window-major tiles]; dense pack
        # per (window, slab).
        key = w * NSLAB + s_slab
        cnts_ws = np.bincount(key, minlength=NW * NSLAB)
        starts = np.zeros(NW * NSLAB, np.int64)
        starts[1:] = np.cumsum(cnts_ws)[:-1]
        pos = np.arange(len(ld)) - starts[key]
        g_ = w // WPG
        b_ = w % WPG
        base_T = g_ * TPG + np.where(s_slab == 0, 0, WPG * TPWs[0]) + \
            b_ * np.where(s_slab == 0, TPWs[0], TPWs[1] if NSLAB > 1 else 0)
        slot = base_T * P + pos

        pidx = np.zeros(SLOTS, np.int64)              # pair row within slab
        oneh = np.zeros((SLOTS, 2 * WIN), ml_dtypes.bfloat16)
        pidx[slot] = s_pair - s_slab * SLAB
        oneh[slot, (ld % WIN) + WIN * s_half] = 1.0

        # idx device layout: per (group, slab) contiguous block,
        # 16-partition wrap, replicated to 128 partitions
        idx_dev = np.zeros((P, T_TOTAL * P // 16), np.int16)
        col = 0
        pidx_t = pidx.reshape(T_TOTAL, P)
        for g in range(NT):
            for s in range(NSLAB):
                t0 = g * TPG + (0 if s == 0 else WPG * TPWs[0])
                blk = pidx_t[t0:t0 + WPG * TPWs[s]].reshape(-1)
                nb = blk.size // 16
                idx_dev[:16, col:col + nb] = blk.reshape(nb, 16).T
                col += nb
        idx_dev[16:, :] = np.tile(idx_dev[:16, :], (7, 1))

        oneh = np.ascontiguousarray(
            oneh.reshape(T_TOTAL, P, 2 * WIN).transpose(1, 0, 2)
        ).reshape(P, T_TOTAL * 2 * WIN)

        # ---- per-node scale vectors, local layout (node l = 128 t + p)
        n_real = int(n1s[c] - n0s[c])
        l_arr = np.arange(NPAD)
        gl = np.minimum(n0s[c] + l_arr, n - 1)
        valid = l_arr < n_real
        dinv_l = np.where(valid, dinv[gl], 0.0).astype(np.float32)
        sdeg_l = np.where(valid, sdeg[gl], 0.0).astype(np.float32)
        dinvp = np.ascontiguousarray(dinv_l.reshape(NT, P).T)
        dinv2p = np.ascontiguousarray((dinv_l ** 2).reshape(NT, P).T)
        sdeg3 = np.zeros((65, NW3 * WIN), np.float32)
        sw = sdeg_l.reshape(NW, WIN)
        for r in range(3):
            rows = sw[r::3]
            sdeg3[32 * r, :rows.shape[0] * WIN] = rows.reshape(-1)

        # ---- pooling one-hot (binary)
        cnt_g = np.bincount((batch[n0s[c]:n1s[c]] - c * GRAPHS_PER_CORE).astype(np.int64),
                            minlength=GRAPHS_PER_CORE).astype(np.float32)
        invc = (1.0 / np.maximum(cnt_g, 1.0)).astype(np.float32).reshape(1, GRAPHS_PER_CORE)
        lg = (batch[gl] - c * GRAPHS_PER_CORE).astype(np.int64)
        poolh = np.zeros((NPAD, GRAPHS_PER_CORE), ml_dtypes.bfloat16)
        poolh[l_arr[valid], lg[valid]] = 1.0
        poolh = np.ascontiguousarray(
            poolh.reshape(NT, P, GRAPHS_PER_CORE).transpose(1, 0, 2)
        ).reshape(P, NT * GRAPHS_PER_CORE)

        in_maps.append({
            "gidx": idx_dev, "oneh": oneh, "poolh": poolh,
            "dinvp": dinvp, "dinv2p": dinv2p, "sdeg3": sdeg3, "invc": invc,
        })

    # ---- replicated tensors: xpack in unified-row order
    # xstage tile (g, j) partition p -> table1 row r = p*XT_TILES + g*XG_TILES + j
    # xpack column (g, j*128 + p) holds node_of_R(r) features (or 0)
    rows_grid = np.arange(P * XT_TILES).reshape(P, XT_TILES)
    p1 = rows_grid // XT_TILES
    i1 = rows_grid % XT_TILES
    # r for (p, i):
    r_of_pi = p1 * XT_TILES + i1   # == rows_grid... identity by construction
    # node for table1 row r (r may exceed NROWS for pad tiles)
    node_pi = np.where(r_of_pi < NROWS, node_of_R[np.minimum(r_of_pi, NROWS - 1)], -1)
    dinv1p = np.where(node_pi >= 0, dinv[np.maximum(node_pi, 0)], 0.0).astype(np.float32)

    xt = np.zeros((84, XG_W), ml_dtypes.bfloat16)
    xT = x.T.astype(ml_dtypes.bfloat16)      # [20, n]
    for g in range(XGROUPS):
        for j in range(XG_TILES):
            i = g * XG_TILES + j
            nodes = node_pi[:, i]             # [128] node per partition
            ok = nodes >= 0
            colsl = slice(j * P, j * P + P)
            blk = np.zeros((IN_DIM, P), ml_dtypes.bfloat16)
            blk[:, ok] = xT[:, nodes[ok]]
            xt[g * 32:g * 32 + IN_DIM, colsl] = blk

    w1rep = np.zeros((84, NODE_DIM), ml_dtypes.bfloat16)
    b1rep = np.zeros((65, NODE_DIM), np.float32)
    b2rep = np.zeros((65, NODE_DIM), np.float32)
    w1_bf = np.asarray(W1, np.float32).astype(ml_dtypes.bfloat16)
    for r in range(3):
        w1rep[32 * r:32 * r + IN_DIM, :] = w1_bf
        b1rep[32 * r, :] = np.asarray(b1, np.float32)
        b2rep[32 * r, :] = np.asarray(b2, np.float32)

    shared = {
        "xpack": xt,
        "ident": np.eye(P, dtype=ml_dtypes.bfloat16),
        "w1rep": w1rep,
        "w2": np.asarray(W2, np.float32).astype(ml_dtypes.bfloat16),
        "wfc": np.asarray(Wfc, np.float32).astype(ml_dtypes.bfloat16),
        "b1rep": b1rep,
        "b2rep": b2rep,
        "bfc": np.full((1, GRAPHS_PER_CORE), np.float32(np.asarray(bfc).reshape(-1)[0])),
        "dinv1p": np.ascontiguousarray(dinv1p),
    }
    for m in in_maps:
        m.update(shared)
    return in_maps, tuple(TPWs)


# ============================================================= device program
def _build_program(TPWs, debug=False, stages=5, repeat=1, parts='all', nq=1):
    TPW_ALL = int(sum(TPWs))
    T_TOTAL = NW * TPW_ALL
    TPG = WPG * TPW_ALL                  # tiles per node-group
    IDXC = T_TOTAL * P // 16             # idx cols total
    OH_GROUPS = 2

    nc = bacc.Bacc(num_swdge_queues=nq)
    xpack = nc.declare_dram_parameter("xpack", [84, XG_W], BF16, isOutput=False)
    ident = nc.declare_dram_parameter("ident", [P, P], BF16, isOutput=False)
    w1rep = nc.declare_dram_parameter("w1rep", [84, NODE_DIM], BF16, isOutput=False)
    w2 = nc.declare_dram_parameter("w2", [NODE_DIM, NODE_DIM], BF16, isOutput=False)
    wfc = nc.declare_dram_parameter("wfc", [NODE_DIM, 1], BF16, isOutput=False)
    b1rep = nc.declare_dram_parameter("b1rep", [65, NODE_DIM], F32, isOutput=False)
    b2rep = nc.declare_dram_parameter("b2rep", [65, NODE_DIM], F32, isOutput=False)
    bfc = nc.declare_dram_parameter("bfc", [1, GRAPHS_PER_CORE], F32, isOutput=False)
    invc = nc.declare_dram_parameter("invc", [1, GRAPHS_PER_CORE], F32, isOutput=False)
    dinv1p = nc.declare_dram_parameter("dinv1p", [P, XT_TILES], F32, isOutput=False)
    dinvp = nc.declare_dram_parameter("dinvp", [P, NT], F32, isOutput=False)
    dinv2p = nc.declare_dram_parameter("dinv2p", [P, NT], F32, isOutput=False)
    sdeg3 = nc.declare_dram_parameter("sdeg3", [65, NW3 * WIN], F32, isOutput=False)
    gidx = nc.declare_dram_parameter("gidx", [P, IDXC], I16, isOutput=False)
    oneh = nc.declare_dram_parameter("oneh", [P, T_TOTAL * 2 * WIN], BF16, isOutput=False)
    poolh = nc.declare_dram_parameter("poolh", [P, NT * GRAPHS_PER_CORE], BF16, isOutput=False)
    out = nc.declare_dram_parameter("out", [1, GRAPHS_PER_CORE], F32, isOutput=True)

    table1 = nc.dram_tensor("table1", [TAB1_ROWS, NODE_DIM], BF16)
    h1_slice = nc.dram_tensor("h1_slice", [NPAD, NODE_DIM], BF16)
    table2 = nc.dram_tensor("table2", [NROWS, NODE_DIM], BF16, addr_space="Shared")
    t1_pair = table1[:].rearrange("(q two) d -> q (two d)", two=2)   # [51264, 128]
    t2_pair = table2[:].rearrange("(q two) d -> q (two d)", two=2)   # [51200, 128]

    if debug:
        dbg_h1 = nc.declare_dram_parameter("dbg_h1", [P, NT * NODE_DIM], BF16, isOutput=True)
        dbg_h2 = nc.declare_dram_parameter("dbg_h2", [P, NT * NODE_DIM], BF16, isOutput=True)
        dbg_gt = nc.declare_dram_parameter("dbg_gt", [P, TPG * 2 * NODE_DIM], BF16, isOutput=True)
        dbg_ps = nc.declare_dram_parameter("dbg_ps", [32, NODE_DIM], F32, isOutput=True)
        dbg_pool = nc.declare_dram_parameter("dbg_pool", [NODE_DIM, GRAPHS_PER_CORE], F32, isOutput=True)

    with tile.TileContext(nc) as tc:
        with (
            tc.tile_pool(name="const", bufs=1) as constp,
            tc.tile_pool(name="idxp", bufs=3) as idxp,
            tc.tile_pool(name="xstr", bufs=2) as xstrp,
            tc.tile_pool(name="stage", bufs=2) as stagep,
            tc.tile_pool(name="gat", bufs=3) as gatp,
            tc.tile_pool(name="ohp", bufs=3) as ohp,
            tc.tile_pool(name="hsb", bufs=1) as hsbp,
            tc.tile_pool(name="php", bufs=2) as php,
            tc.tile_pool(name="psA", bufs=1, space="PSUM") as psA,
            tc.tile_pool(name="psB", bufs=3, space="PSUM") as psB,
            tc.tile_pool(name="psT", bufs=2, space="PSUM") as psT,
            tc.tile_pool(name="psC", bufs=1, space="PSUM") as psC,
        ):
            # ---------------- constants
            w1_sb = constp.tile([84, NODE_DIM], BF16)
            w2_sb = constp.tile([NODE_DIM, NODE_DIM], BF16)
            wfc_sb = constp.tile([NODE_DIM, 1], BF16)
            b1_sb = constp.tile([65, NODE_DIM], F32)
            b2_sb = constp.tile([65, NODE_DIM], F32)
            bfc_sb = constp.tile([1, GRAPHS_PER_CORE], F32)
            invc_sb = constp.tile([1, GRAPHS_PER_CORE], F32)
            dinv1_sb = constp.tile([P, XT_TILES], F32)
            dinv_sb = constp.tile([P, NT], F32)
            dinv2_sb = constp.tile([P, NT], F32)
            sdeg_sb = constp.tile([65, NW3 * WIN], F32)
            id_sb = constp.tile([P, P], BF16)
            nc.sync.dma_start(out=id_sb[:], in_=ident[:])
            for dst_t, src_t in ((w1_sb, w1rep), (w2_sb, w2), (wfc_sb, wfc),
                                 (b1_sb, b1rep), (b2_sb, b2rep), (bfc_sb, bfc),
                                 (invc_sb, invc), (dinv1_sb, dinv1p),
                                 (dinv_sb, dinvp), (dinv2_sb, dinv2p),
                                 (sdeg_sb, sdeg3)):
                nc.sync.dma_start(out=dst_t[:], in_=src_t[:])

            # -------- stage X: table1 = dinv * (x @ W1), bf16, unified rows
            t1v = table1[:].rearrange("(p t) d -> p (t d)", p=P)
            for j2 in (range(0, XG_TILES, XCHUNK) if stages >= 1 else []):
                jn = min(XCHUNK, XG_TILES - j2)
                xp_sb = xstrp.tile([84, XCHUNK * P], BF16, tag="xp")
                nc.sync.dma_start(out=xp_sb[:, :jn * P],
                                  in_=xpack[:, j2 * P:(j2 + jn) * P])
                for g in range(XGROUPS):
                    stg = stagep.tile([P, XCHUNK * NODE_DIM], BF16, tag="xstg")
                    for j3 in range(0, jn, 8):
                        j4 = min(8, jn - j3)
                        ps = psA.tile([P, 8 * NODE_DIM], F32, tag="xps")
                        for j in range(j3, j3 + j4):
                            nc.tensor.matmul(
                                out=ps[:, (j - j3) * NODE_DIM:(j - j3 + 1) * NODE_DIM],
                                lhsT=xp_sb[g * 32:g * 32 + IN_DIM, j * P:(j + 1) * P],
                                rhs=w1_sb[g * 32:g * 32 + IN_DIM, :],
                                start=True, stop=True,
                            )
                        ti = g * XG_TILES + j2 + j3
                        nc.vector.tensor_tensor(
                            out=stg[:, j3 * NODE_DIM:(j3 + j4) * NODE_DIM]
                                .rearrange("p (a b) -> p a b", b=NODE_DIM),
                            in0=ps[:, :j4 * NODE_DIM]
                                .rearrange("p (a b) -> p a b", b=NODE_DIM),
                            in1=dinv1_sb[:, ti:ti + j4].to_broadcast([P, j4, NODE_DIM]),
                            op=mybir.AluOpType.mult,
                        )
                    nc.sync.dma_start(
                        out=t1v[:, (g * XG_TILES + j2) * NODE_DIM:
                                (g * XG_TILES + j2 + jn) * NODE_DIM],
                        in_=stg[:, :jn * NODE_DIM],
                    )

            # table1 writes must complete before layer-1 gathers read it
            tc.strict_bb_all_engine_barrier()

            # ---------------- message-passing layers
            # per-group gather sizes (group = 4 windows)
            n_idx_s = [WPG * TPWs[s] * P for s in range(NSLAB)]   # idxs per (grp, slab)
            idxcols_grp = sum(n_idx_s) // 16
            h_sb = {}
            layer_list = [l for l in (1, 2) if stages >= (2 if l == 1 else 4)] * repeat
            for layer in layer_list:
                table_p = t1_pair if layer == 1 else t2_pair
                bias_sb = b1_sb if layer == 1 else b2_sb
                scale_sb = dinv2_sb if layer == 1 else dinv_sb
                h = hsbp.tile([P, NT * NODE_DIM], BF16, tag=f"h{layer}")
                h_sb[layer] = h

                if parts == 'gather_only':
                    nc.vector.memset(h[:], 0.0)
                static_gt = None
                if parts == 'mm_only':
                    static_gt = gatp.tile([P, TPG * 2 * NODE_DIM], BF16, tag="gt")
                    nc.vector.memset(static_gt[:], 0.0)
                for og in range(0, NT, OH_GROUPS):
                    ogn = min(OH_GROUPS, NT - og)
                    oh_sb = ohp.tile([P, OH_GROUPS * TPG * 2 * WIN], BF16, tag="oh")
                    nc.sync.dma_start(
                        out=oh_sb[:, :ogn * TPG * 2 * WIN],
                        in_=oneh[:, og * TPG * 2 * WIN:(og + ogn) * TPG * 2 * WIN],
                    )
                    ix_sb = idxp.tile([P, OH_GROUPS * idxcols_grp], I16, tag="ix")
                    nc.sync.dma_start(
                        out=ix_sb[:, :ogn * idxcols_grp],
                        in_=gidx[:, og * idxcols_grp:(og + ogn) * idxcols_grp],
                    )
                    for bg in range(og, og + ogn):
                        # gather: one call per slab; tile layout within group:
                        # [slab0: WPG*TPW0 tiles][slab1: WPG*TPW1 tiles]
                        gt = static_gt if parts == 'mm_only' else \
                            gatp.tile([P, TPG * 2 * NODE_DIM], BF16, tag="gt")
                        gtv = gt[:].rearrange("p (t r) -> p t r", r=2 * NODE_DIM)
                        colb = (bg - og) * idxcols_grp
                        tile0 = 0
                        for s in (range(NSLAB) if parts in ('all', 'gather_only') else []):
                            nts = WPG * TPWs[s]
                            nc.gpsimd.dma_gather(
                                out_ap=gtv[:, tile0:tile0 + nts, :],
                                in_ap=table_p[s * SLAB:
                                              min((s + 1) * SLAB, table_p.shape[0]), :],
                                idxs_ap=ix_sb[:, colb:colb + n_idx_s[s] // 16],
                                num_idxs=n_idx_s[s],
                                num_idxs_reg=n_idx_s[s],
                                elem_size=2 * NODE_DIM,
                                single_packet=False,
                                queue_num=(bg * NSLAB + s) % nq,
                            )
                            colb += n_idx_s[s] // 16
                            tile0 += nts
                        for b_ in (range(WPG) if parts in ('all', 'mm_only') else []):
                            w_glob = bg * WPG + b_
                            p3, c3 = 32 * (w_glob % 3), w_glob // 3
                            # this window's tiles: slab0 then slab1 segments
                            tlist = [b_ * TPWs[0] + q for q in range(TPWs[0])]
                            if NSLAB > 1:
                                tlist += [WPG * TPWs[0] + b_ * TPWs[1] + q
                                          for q in range(TPWs[1])]
                            ps = psB.tile([32, NODE_DIM], F32, tag="mps")
                            nc.tensor.matmul(
                                out=ps[:],
                                lhsT=sdeg_sb[p3:p3 + 1, c3 * WIN:(c3 + 1) * WIN],
                                rhs=bias_sb[p3:p3 + 1, :], start=True, stop=False)
                            for ti, t in enumerate(tlist):
                                ohb = ((bg - og) * TPG + t) * 2 * WIN
                                for hf in range(2):
                                    nc.tensor.matmul(
                                        out=ps[:],
                                        lhsT=oh_sb[:, ohb + hf * WIN:ohb + (hf + 1) * WIN],
                                        rhs=gt[:, t * 2 * NODE_DIM + hf * NODE_DIM:
                                               t * 2 * NODE_DIM + (hf + 1) * NODE_DIM],
                                        start=False,
                                        stop=(ti == len(tlist) - 1 and hf == 1),
                                    )
                            nc.scalar.activation(
                                out=h[b_ * WIN:(b_ + 1) * WIN,
                                      bg * NODE_DIM:(bg + 1) * NODE_DIM],
                                in_=ps[:], func=mybir.ActivationFunctionType.Relu,
                                scale=scale_sb[b_ * WIN:(b_ + 1) * WIN, bg:bg + 1],
                            )
                        if debug and layer == 1 and bg == 0:
                            nc.sync.dma_start(out=dbg_gt[:], in_=gt[:])

                if layer == 1 and stages >= 3:
                    # table2 rows must be dinv*(h1 @ W2): conv2 = (A_hat h1) W2
                    # commutes, so transform the slice before the AllGather.
                    # Per node-group: transpose -> W2 matmul -> transpose back.
                    h1w = hsbp.tile([P, NT * NODE_DIM], BF16, tag="h1w")
                    for g in range(NT):
                        psT1 = psT.tile([NODE_DIM, P], BF16, tag="tr")
                        nc.tensor.transpose(
                            out=psT1[:], in_=h[:, g * NODE_DIM:(g + 1) * NODE_DIM],
                            identity=id_sb[:])
                        hT = stagep.tile([NODE_DIM, P], BF16, tag="hT")
                        nc.vector.tensor_copy(out=hT[:], in_=psT1[:])
                        psT2 = psT.tile([NODE_DIM, P], F32, tag="tr")
                        nc.tensor.matmul(out=psT2[:], lhsT=w2_sb[:], rhs=hT[:],
                                         start=True, stop=True)
                        hwT = stagep.tile([NODE_DIM, P], BF16, tag="hwT")
                        nc.vector.tensor_copy(out=hwT[:], in_=psT2[:])
                        psT3 = psT.tile([P, NODE_DIM], BF16, tag="tr")
                        nc.tensor.transpose(
                            out=psT3[:], in_=hwT[:], identity=id_sb[:NODE_DIM, :NODE_DIM])
                        nc.scalar.copy(
                            out=h1w[:, g * NODE_DIM:(g + 1) * NODE_DIM], in_=psT3[:])
                    nc.sync.dma_start(
                        out=h1_slice[:].rearrange("(p t) d -> p (t d)", p=P),
                        in_=h1w[:])
                    nc.gpsimd.collective_compute(
                        "AllGather",
                        mybir.AluOpType.bypass,
                        replica_groups=[list(range(N_CORES))],
                        ins=[h1_slice[:]],
                        outs=[table2[:]],
                    )
                    tc.strict_bb_all_engine_barrier()
                    if debug:
                        nc.sync.dma_start(out=dbg_h1[:], in_=h[:])
                if layer == 2 and debug:
                    nc.sync.dma_start(out=dbg_h2[:], in_=h[:])

            # ---------------- mean pool + fc
            if stages < 5:
                zo = stagep.tile([1, GRAPHS_PER_CORE], F32, tag="osb")
                nc.vector.memset(zo[:], 0.0)
                nc.sync.dma_start(out=out[:], in_=zo[:])
            pool_ps = psC.tile([NODE_DIM, GRAPHS_PER_CORE], F32, tag="pps")
            PHC = 25
            for t0 in (range(0, NT, PHC) if stages >= 5 else []):
                ph_sb = php.tile([P, PHC * GRAPHS_PER_CORE], BF16, tag="ph")
                nc.sync.dma_start(
                    out=ph_sb[:],
                    in_=poolh[:, t0 * GRAPHS_PER_CORE:(t0 + PHC) * GRAPHS_PER_CORE])
                for t in range(t0, t0 + PHC):
                    nc.tensor.matmul(
                        out=pool_ps[:],
                        lhsT=h_sb[2][:, t * NODE_DIM:(t + 1) * NODE_DIM],
                        rhs=ph_sb[:, (t - t0) * GRAPHS_PER_CORE:
                                  (t - t0 + 1) * GRAPHS_PER_CORE],
                        start=(t == 0), stop=(t == NT - 1),
                    )
            pool_sb = stagep.tile([NODE_DIM, GRAPHS_PER_CORE], BF16, tag="pool")
            if stages >= 5:
                nc.vector.tensor_copy(out=pool_sb[:], in_=pool_ps[:])
            if debug:
                pool_f32 = stagep.tile([NODE_DIM, GRAPHS_PER_CORE], F32, tag="poolf")
                nc.vector.tensor_copy(out=pool_f32[:], in_=pool_ps[:])
                nc.sync.dma_start(out=dbg_pool[:], in_=pool_f32[:])

            if stages >= 5:
                fc_ps = psC.tile([1, GRAPHS_PER_CORE], F32, tag="fc")
                nc.tensor.matmul(out=fc_ps[:], lhsT=wfc_sb[:], rhs=pool_sb[:],
                                 start=True, stop=True)
                out_sb = stagep.tile([1, GRAPHS_PER_CORE], F32, tag="osb")
                nc.vector.tensor_tensor(out=out_sb[:], in0=fc_ps[:], in1=invc_sb[:],
                                        op=mybir.AluOpType.mult)
                nc.vector.tensor_tensor(out=out_sb[:], in0=out_sb[:], in1=bfc_sb[:],
                                        op=mybir.AluOpType.add)
                nc.sync.dma_start(out=out[:], in_=out_sb[:])

    nc.compile()
    return nc


# ================================================================== kernel
def kernel(**inputs) -> np.ndarray:
    in_maps, TPWs = _preprocess(
        inputs["x"], inputs["edge_index"], inputs["batch"],
        inputs["W1"], inputs["b1"], inputs["W2"], inputs["b2"],
        inputs["Wfc"], inputs["bfc"],
    )
    if TPWs not in _CACHE:
        _CACHE[TPWs] = _build_program(TPWs)
    nc = _CACHE[TPWs]
    res = run_bass_kernel_spmd(nc, in_maps, list(range(N_CORES)))
    outs = [res.results[c]["out"].reshape(-1) for c in range(N_CORES)]
    return np.concatenate(outs).astype(np.float32)
